# revision 10
# baseline (speedup 1.0000x reference)
"""Trainium2 Bass kernel for nn_LocalRNN: 8-step CTRNN over sliding windows.

Math:
  For each position l: h_{k+1} = a*h_k + relu(h_k @ W* + u*[l+k]),  h_0 = 0
  where a = 1 - 1/tau, W* = W * (1/tau) (columns), u* = Xp @ W_in* + b*,
  W_in* = W_in * (1/tau), b* = b * (1/tau).  Output = h_8 per position.
  (Uses relu(c*z) = c*relu(z) for c>0 to fold 1/tau into the weights, and
  the fact that the input projection is shared across overlapping windows.)

Sharding: batch dim (8) across the 8 NeuronCores, weights replicated.
On-chip layout is transposed ([d on partitions, positions on free dim]) so
matmuls contract d on the partition axis; the host uploads x pre-transposed
and swizzled, and un-swizzles the chunk-major output (layout marshalling
only, off the measured path).

All matmul/DVE operands are fp16 (PSUM accumulation stays fp32): enables
FWL weight loads on PE, 2x DVE mode for the recurrence, and halves DMA.
"""

import numpy as np
from contextlib import ExitStack

import concourse.bass as bass
import concourse.tile as tile
from concourse import bacc, mybir
from concourse.bass_utils import run_bass_kernel_spmd

B, L, D, KSIZE = 8, 2048, 256, 8
P = 128
NCORES = 8
MMN = 512                    # matmul moving free dim (PSUM bank limit)
WCH = 1024                   # wide chunk for ACT/DVE elementwise ops
NW = L // WCH                # 2
NG = L // MMN                # 4 groups of 512
UCOLS = L + KSIZE - 1        # 2055
PAD = KSIZE - 1              # 7
DB = D // P                  # 2 d-blocks
F32 = mybir.dt.float32
F16 = mybir.dt.float16
AF = mybir.ActivationFunctionType
ALU = mybir.AluOpType

# fp16 weights blob: wint0|wint1|wt0|wt1|identr
WB_COLS = 4 * D + P
WB_ID = 4 * D
# fp32 consts blob: bst (DB) | at (DB) | spare
CF_COLS = 16
N_WARM = 20
_cache = {}


def _build_program():
    nc = bacc.Bacc(
        "TRN2",
        target_bir_lowering=False,
        debug=False,
        enable_asserts=False,
        num_devices=NCORES,
    )
    # x uploaded pre-transposed+swizzled: row p, cols (i, l): x[l, i*128+p]
    x_d = nc.dram_tensor("xt", (P, DB * L), F16, kind="ExternalInput").ap()
    wb_d = nc.dram_tensor("wblob", (P, WB_COLS), F16, kind="ExternalInput").ap()
    cf_d = nc.dram_tensor("constsf", (P, CF_COLS), F32, kind="ExternalInput").ap()
    # output chunk-major: rows (c, p), cols (i, s): h8[i*128+p, c*512+s]
    out_d = nc.dram_tensor("out", (NG * P, DB * MMN), F16, kind="ExternalOutput").ap()

    with tile.TileContext(nc) as tc, ExitStack() as ctx:
        consts = ctx.enter_context(tc.tile_pool(name="consts", bufs=1))
        big = ctx.enter_context(tc.tile_pool(name="big", bufs=1))
        rp = ctx.enter_context(tc.tile_pool(name="rp", bufs=3))
        app = ctx.enter_context(tc.tile_pool(name="app", bufs=8))
        # single PSUM pool, all tags share slots: [128,1024] slot = 2 banks,
        # bufs=4 -> 8 banks
        zp = ctx.enter_context(tc.tile_pool(name="zp", bufs=4, space="PSUM"))

        # --- PE warmup on a memset dummy: starts right after the preamble
        # (no DMA dependency) and rides the HAM ramp while inputs land.
        dummy = big.tile([P, MMN], F16, name="dummy")
        nc.vector.memset(dummy[:], 0.0)
        warm = zp.tile([P, MMN], F32, name="warm", tag="z")
        for _ in range(N_WARM):
            nc.tensor.matmul(warm[:], lhsT=dummy[:, 0:P], rhs=dummy[:],
                             start=True, stop=True)

        # --- constants ---
        wb = consts.tile([P, WB_COLS], F16, name="wb")
        cf = consts.tile([P, CF_COLS], F32, name="cf")
        wint = [wb[:, i * D:(i + 1) * D] for i in range(DB)]
        wt = [wb[:, 2 * D + i * D:2 * D + (i + 1) * D] for i in range(DB)]
        identr = wb[:, WB_ID:WB_ID + P]
        bst = cf[:, 0:DB]
        at = cf[:, DB:2 * DB]

        # --- persistent buffers ---
        # x in 2 per-position-half tiles (both d-blocks each) so the first
        # half of the u projection only waits on one 0.5MB DMA
        xth = [big.tile([P, DB * WCH], F16, name=f"xth{g}") for g in range(2)]
        ut = [big.tile([P, UCOLS], F16, name=f"ut{i}") for i in range(DB)]
        hball = [big.tile([P, DB * L], F16, name=f"hb{s}") for s in range(2)]
        hb = [[hball[s][:, i * L:(i + 1) * L] for i in range(DB)]
              for s in range(2)]
        h1 = hb[1]

        # --- input DMAs. Only sync+scalar have HWDGE; each dma_start costs
        # ~800ns of descriptor-gen on its sequencer, so spread and order by
        # need: weights first (PE warmup), then cf, then x halves.
        def xdma(eng, g):
            eng.dma_start(
                xth[g][:].rearrange("p (i c) -> p i c", i=DB),
                x_d.rearrange("p (i c) -> p i c", i=DB)[
                    :, :, g * WCH:(g + 1) * WCH],
            )
        nc.sync.dma_start(wb[:], wb_d[:, :])
        nc.scalar.dma_start(cf[:], cf_d[:, :])
        xdma(nc.sync, 0)
        xdma(nc.scalar, 1)

        # u pad cols + h1 pad cols (also warms the ACT table early):
        # u[:, :7] = b*, h1[:, :7] = relu(b*)
        for j in range(DB):
            nc.scalar.activation(
                ut[j][:, 0:PAD], cf[:, 0:PAD],
                AF.Identity, bias=bst[:, j:j + 1], scale=0.0,
            )
            nc.scalar.activation(
                h1[j][:, 0:PAD], cf[:, 0:PAD],
                AF.Relu, bias=bst[:, j:j + 1], scale=0.0,
            )

        # --- u projection, wide tiles; h1 (ACT) and u (DVE) read PSUM ---
        for gw in range(2):
            for j in range(DB):
                zt = zp.tile([P, WCH], F32, name="zu", tag="z")
                for half in range(2):
                    g = 2 * gw + half
                    zh = zt[:, half * MMN:(half + 1) * MMN]
                    for i in range(DB):
                        nc.tensor.matmul(
                            zh,
                            lhsT=wint[i][:, j * P:(j + 1) * P],
                            rhs=xth[gw][:, i * WCH + half * MMN:
                                        i * WCH + half * MMN + MMN],
                            start=(i == 0),
                            stop=(i == DB - 1),
                        )
                # h1 positions [7+1024gw, min(7+1024(gw+1), 2048))
                hw = WCH if gw == 0 else WCH - PAD
                nc.scalar.activation(
                    h1[j][:, PAD + gw * WCH:PAD + gw * WCH + hw],
                    zt[:, 0:hw], AF.Relu, bias=bst[:, j:j + 1], scale=1.0,
                )
                nc.vector.tensor_scalar(
                    out=ut[j][:, PAD + gw * WCH:PAD + (gw + 1) * WCH],
                    in0=zt[:],
                    scalar1=bst[:, j:j + 1],
                    scalar2=None,
                    op0=ALU.add,
                )

        # --- steps 1..6 (wide 1024-col chunks; matmuls in 512 halves) ---
        for k in range(1, KSIZE - 1):
            hc = hb[k % 2]
            hn = hb[(k + 1) % 2]
            # a*h precomputed on DVE (tensor_scalar runs 4x on fp16 SBUF)
            # while PE does the step's matmuls; the post-relu combine is a
            # 2x tensor_tensor. scalar_tensor_tensor would run 1x.
            ahs = {}
            for c in range(NW):
                for j in range(DB):
                    ah = app.tile([P, WCH], F16, name="ah", tag="ah")
                    nc.vector.tensor_scalar(
                        out=ah[:],
                        in0=hc[j][:, c * WCH:(c + 1) * WCH],
                        scalar1=at[:, j:j + 1],
                        scalar2=None,
                        op0=ALU.mult,
                    )
                    ahs[c, j] = ah
            for c in range(NW):
                cs = c * WCH
                for j in range(DB):
                    zt = zp.tile([P, WCH], F32, name="zt", tag="z")
                    for h in range(2):
                        hs = cs + h * MMN
                        zh = zt[:, h * MMN:(h + 1) * MMN]
                        for i in range(DB):
                            nc.tensor.matmul(
                                zh,
                                lhsT=wt[i][:, j * P:(j + 1) * P],
                                rhs=hc[i][:, hs:hs + MMN],
                                start=(i == 0),
                                stop=False,
                            )
                        nc.tensor.matmul(
                            zh,
                            lhsT=identr,
                            rhs=ut[j][:, k + hs:k + hs + MMN],
                            start=False,
                            stop=True,
                        )
                    r = rp.tile([P, WCH], F16, name="r", tag="r")
                    nc.scalar.activation(r[:], zt[:], AF.Relu)
                    nc.vector.tensor_tensor(
                        out=hn[j][:, cs:cs + WCH],
                        in0=ahs[c, j][:],
                        in1=r[:],
                        op=ALU.add,
                    )

        # --- step 7 in 512-col chunks, output DMA per chunk ---
        k = KSIZE - 1
        hc = hb[k % 2]
        hn = hb[(k + 1) % 2]
        h8all = hball[(k + 1) % 2]
        ahs7 = {}
        for g in range(NG):
            for j in range(DB):
                ah = app.tile([P, MMN], F16, name="ah7", tag="ah")
                nc.vector.tensor_scalar(
                    out=ah[:],
                    in0=hc[j][:, g * MMN:(g + 1) * MMN],
                    scalar1=at[:, j:j + 1],
                    scalar2=None,
                    op0=ALU.mult,
                )
                ahs7[g, j] = ah
        # last 512-chunk tapers into 2x256 with descriptor-gens on different
        # engines, shortening the final relu->tt->dma drain chain.
        pieces = [(0, 0, MMN, nc.sync), (1, 0, MMN, nc.scalar),
                  (2, 0, MMN, nc.sync), (3, 0, MMN // 2, nc.sync),
                  (3, MMN // 2, MMN // 2, nc.scalar)]
        for g, off, w, eng in pieces:
            cs = g * MMN + off
            for j in range(DB):
                zt = zp.tile([P, w], F32, name="z7", tag="z")
                for i in range(DB):
                    nc.tensor.matmul(
                        zt[:],
                        lhsT=wt[i][:, j * P:(j + 1) * P],
                        rhs=hc[i][:, cs:cs + w],
                        start=(i == 0),
                        stop=False,
                    )
                nc.tensor.matmul(
                    zt[:],
                    lhsT=identr,
                    rhs=ut[j][:, k + cs:k + cs + w],
                    start=False,
                    stop=True,
                )
                r = rp.tile([P, w], F16, name="r7", tag="r")
                nc.scalar.activation(r[:], zt[:], AF.Relu)
                nc.vector.tensor_tensor(
                    out=hn[j][:, cs:cs + w],
                    in0=ahs7[g, j][:, off:off + w],
                    in1=r[:],
                    op=ALU.add,
                )
            # chunk-major store: rows [g*128, (g+1)*128) of out_d.
            # sync is idle in steady state; keep scalar free for relus.
            eng.dma_start(
                out_d.rearrange("(c p) f -> c p f", p=P)[g]
                     .rearrange("p (i s) -> p i s", i=DB)[:, :, off:off + w],
                h8all[:].rearrange("p (i c) -> p i c", i=DB)[
                    :, :, cs:cs + w],
            )

    nc.compile()
    return nc


def get_program():
    if "nc" not in _cache:
        _cache["nc"] = _build_program()
    return _cache["nc"]


def make_in_maps(x, weight, input_weight, bias, tau):
    x = np.asarray(x, dtype=np.float32)
    weight = np.asarray(weight, dtype=np.float32)
    input_weight = np.asarray(input_weight, dtype=np.float32)
    bias = np.asarray(bias, dtype=np.float32).reshape(1, D)
    tau = np.asarray(tau, dtype=np.float32).reshape(1, D)

    inv_tau = 1.0 / tau                       # (1, D)
    a = 1.0 - inv_tau
    wstar = (weight * inv_tau).astype(np.float32)          # scale columns
    winstar = (input_weight * inv_tau).astype(np.float32)
    bstar = (bias * inv_tau).astype(np.float32)
    # per-partition layout (P, DB): col j holds elems [j*P, (j+1)*P)
    bstar_t = bstar.reshape(DB, P).T
    a_t = a.reshape(DB, P).T
    ident = np.eye(P, dtype=np.float32)

    wb = np.concatenate(
        [winstar[0:P, :], winstar[P:D, :], wstar[0:P, :], wstar[P:D, :],
         ident], axis=1).astype(np.float16)
    cf = np.zeros((P, CF_COLS), np.float32)
    cf[:, 0:DB] = bstar_t
    cf[:, DB:2 * DB] = a_t

    shared = {
        "wblob": np.ascontiguousarray(wb),
        "constsf": np.ascontiguousarray(cf),
    }
    ins = []
    for b in range(NCORES):
        # xt[p, i*L + l] = x[b][l, i*128+p]
        xt = np.ascontiguousarray(
            x[b].T.reshape(DB, P, L).transpose(1, 0, 2).reshape(P, DB * L)
            .astype(np.float16))
        ins.append({"xt": xt, **shared})
    return ins


def kernel(x, weight, input_weight, bias, tau, ksize, _trace=False):
    assert int(ksize) == KSIZE
    nc = get_program()
    in_maps = make_in_maps(x, weight, input_weight, bias, tau)
    res = run_bass_kernel_spmd(
        nc, in_maps, core_ids=list(range(NCORES)), trace=_trace
    )
    outs = []
    for b in range(NCORES):
        od = np.asarray(res.results[b]["out"])  # (NG*P, DB*MMN) f16
        out_b = (od.reshape(NG, P, DB, MMN).transpose(0, 3, 2, 1)
                 .reshape(L, D))
        outs.append(out_b)
    out = np.stack(outs, axis=0)
    if _trace:
        _cache["last_results"] = res
    return out.astype(np.float32)


# revision 12
# speedup vs baseline: 1.0093x; 1.0093x over previous
"""Trainium2 Bass kernel for nn_LocalRNN: 8-step CTRNN over sliding windows.

Math:
  For each position l: h_{k+1} = a*h_k + relu(h_k @ W* + u*[l+k]),  h_0 = 0
  where a = 1 - 1/tau, W* = W * (1/tau) (columns), u* = Xp @ W_in* + b*,
  W_in* = W_in * (1/tau), b* = b * (1/tau).  Output = h_8 per position.
  (Uses relu(c*z) = c*relu(z) for c>0 to fold 1/tau into the weights, and
  the fact that the input projection is shared across overlapping windows.)

Sharding: batch dim (8) across the 8 NeuronCores, weights replicated.
On-chip layout is transposed ([d on partitions, positions on free dim]) so
matmuls contract d on the partition axis; the host uploads x pre-transposed
and swizzled, and un-swizzles the chunk-major output (layout marshalling
only, off the measured path).

All matmul/DVE operands are fp16 (PSUM accumulation stays fp32): enables
FWL weight loads on PE, 2x DVE mode for the recurrence, and halves DMA.
"""

import numpy as np
from contextlib import ExitStack

import concourse.bass as bass
import concourse.tile as tile
from concourse import bacc, mybir
from concourse.bass_utils import run_bass_kernel_spmd

B, L, D, KSIZE = 8, 2048, 256, 8
P = 128
NCORES = 8
MMN = 512                    # matmul moving free dim (PSUM bank limit)
WCH = 1024                   # wide chunk for ACT/DVE elementwise ops
NW = L // WCH                # 2
NG = L // MMN                # 4 groups of 512
UCOLS = L + KSIZE - 1        # 2055
PAD = KSIZE - 1              # 7
DB = D // P                  # 2 d-blocks
F32 = mybir.dt.float32
F16 = mybir.dt.float16
AF = mybir.ActivationFunctionType
ALU = mybir.AluOpType

# fp16 weights blob: wint0|wint1|wt0|wt1|identr
WB_COLS = 4 * D + P
WB_ID = 4 * D
# fp32 consts blob: bst (DB) | at (DB) | spare
CF_COLS = 16
N_WARM = 18
_cache = {}


def _build_program():
    nc = bacc.Bacc(
        "TRN2",
        target_bir_lowering=False,
        debug=False,
        enable_asserts=False,
        num_devices=NCORES,
    )
    # x uploaded pre-transposed+swizzled: row p, cols (i, l): x[l, i*128+p]
    x_d = nc.dram_tensor("xt", (P, DB * L), F16, kind="ExternalInput").ap()
    wb_d = nc.dram_tensor("wblob", (P, WB_COLS), F16, kind="ExternalInput").ap()
    cf_d = nc.dram_tensor("constsf", (P, CF_COLS), F32, kind="ExternalInput").ap()
    # output chunk-major: rows (c, p), cols (i, s): h8[i*128+p, c*512+s]
    out_d = nc.dram_tensor("out", (NG * P, DB * MMN), F16, kind="ExternalOutput").ap()

    with tile.TileContext(nc) as tc, ExitStack() as ctx:
        consts = ctx.enter_context(tc.tile_pool(name="consts", bufs=1))
        big = ctx.enter_context(tc.tile_pool(name="big", bufs=1))
        rp = ctx.enter_context(tc.tile_pool(name="rp", bufs=3))
        app = ctx.enter_context(tc.tile_pool(name="app", bufs=8))
        # single PSUM pool, all tags share slots: [128,1024] slot = 2 banks,
        # bufs=4 -> 8 banks
        zp = ctx.enter_context(tc.tile_pool(name="zp", bufs=4, space="PSUM"))

        # --- PE warmup on a memset dummy: starts right after the preamble
        # (no DMA dependency) and rides the HAM ramp while inputs land.
        dummy = big.tile([P, MMN], F16, name="dummy")
        nc.vector.memset(dummy[:], 0.0)
        warm = zp.tile([P, MMN], F32, name="warm", tag="z")
        for _ in range(N_WARM):
            nc.tensor.matmul(warm[:], lhsT=dummy[:, 0:P], rhs=dummy[:],
                             start=True, stop=True)

        # --- constants ---
        wb = consts.tile([P, WB_COLS], F16, name="wb")
        cf = consts.tile([P, CF_COLS], F32, name="cf")
        wint = [wb[:, i * D:(i + 1) * D] for i in range(DB)]
        wt = [wb[:, 2 * D + i * D:2 * D + (i + 1) * D] for i in range(DB)]
        identr = wb[:, WB_ID:WB_ID + P]
        bst = cf[:, 0:DB]
        at = cf[:, DB:2 * DB]

        # --- persistent buffers ---
        # x in 2 per-position-half tiles (both d-blocks each) so the first
        # half of the u projection only waits on one 0.5MB DMA
        xth = [big.tile([P, DB * WCH], F16, name=f"xth{g}") for g in range(2)]
        ut = [big.tile([P, UCOLS], F16, name=f"ut{i}") for i in range(DB)]
        hball = [big.tile([P, DB * L], F16, name=f"hb{s}") for s in range(2)]
        hb = [[hball[s][:, i * L:(i + 1) * L] for i in range(DB)]
              for s in range(2)]
        h1 = hb[1]

        # --- input DMAs. Only sync+scalar have HWDGE; each dma_start costs
        # ~800ns of descriptor-gen on its sequencer, so spread and order by
        # need: weights first (PE warmup), then cf, then x halves.
        def xdma(eng, g):
            eng.dma_start(
                xth[g][:].rearrange("p (i c) -> p i c", i=DB),
                x_d.rearrange("p (i c) -> p i c", i=DB)[
                    :, :, g * WCH:(g + 1) * WCH],
            )
        nc.sync.dma_start(wb[:], wb_d[:, :])
        nc.scalar.dma_start(cf[:], cf_d[:, :])
        xdma(nc.sync, 0)
        xdma(nc.scalar, 1)

        # u pad cols + h1 pad cols (also warms the ACT table early):
        # u[:, :7] = b*, h1[:, :7] = relu(b*)
        for j in range(DB):
            nc.scalar.activation(
                ut[j][:, 0:PAD], cf[:, 0:PAD],
                AF.Identity, bias=bst[:, j:j + 1], scale=0.0,
            )
            nc.scalar.activation(
                h1[j][:, 0:PAD], cf[:, 0:PAD],
                AF.Relu, bias=bst[:, j:j + 1], scale=0.0,
            )

        # --- u projection, wide tiles; h1 (ACT) and u (DVE) read PSUM ---
        for gw in range(2):
            for j in range(DB):
                zt = zp.tile([P, WCH], F32, name="zu", tag="z")
                for half in range(2):
                    g = 2 * gw + half
                    zh = zt[:, half * MMN:(half + 1) * MMN]
                    for i in range(DB):
                        nc.tensor.matmul(
                            zh,
                            lhsT=wint[i][:, j * P:(j + 1) * P],
                            rhs=xth[gw][:, i * WCH + half * MMN:
                                        i * WCH + half * MMN + MMN],
                            start=(i == 0),
                            stop=(i == DB - 1),
                        )
                # h1 positions [7+1024gw, min(7+1024(gw+1), 2048))
                hw = WCH if gw == 0 else WCH - PAD
                nc.scalar.activation(
                    h1[j][:, PAD + gw * WCH:PAD + gw * WCH + hw],
                    zt[:, 0:hw], AF.Relu, bias=bst[:, j:j + 1], scale=1.0,
                )
                nc.vector.tensor_scalar(
                    out=ut[j][:, PAD + gw * WCH:PAD + (gw + 1) * WCH],
                    in0=zt[:],
                    scalar1=bst[:, j:j + 1],
                    scalar2=None,
                    op0=ALU.add,
                )

        # --- steps 1..6 (wide chunks; matmuls in <=512 sub-tiles). Step 6
        # tapers its tail (1024/768/256) so the final two-step drain chain
        # into step 7's last piece runs on short tiles.
        def step_groups(k):
            if k < KSIZE - 2:
                return [(c * WCH, WCH) for c in range(NW)]
            return [(0, WCH), (WCH, 768), (WCH + 768, 256)]

        def mm_halves(w):
            out, off = [], 0
            while off < w:
                m = min(MMN, w - off)
                out.append((off, m))
                off += m
            return out

        for k in range(1, KSIZE - 1):
            hc = hb[k % 2]
            hn = hb[(k + 1) % 2]
            groups = step_groups(k)
            # a*h precomputed on DVE (tensor_scalar runs 4x on fp16 SBUF)
            # while PE does the step's matmuls; the post-relu combine is a
            # 2x tensor_tensor. scalar_tensor_tensor would run 1x.
            ahs = {}
            for gi, (cs, w) in enumerate(groups):
                for j in range(DB):
                    ah = app.tile([P, w], F16, name="ah", tag="ah")
                    nc.vector.tensor_scalar(
                        out=ah[:],
                        in0=hc[j][:, cs:cs + w],
                        scalar1=at[:, j:j + 1],
                        scalar2=None,
                        op0=ALU.mult,
                    )
                    ahs[gi, j] = ah
            for gi, (cs, w) in enumerate(groups):
                for j in range(DB):
                    zt = zp.tile([P, w], F32, name="zt", tag="z")
                    for hs0, hw in mm_halves(w):
                        hs = cs + hs0
                        zh = zt[:, hs0:hs0 + hw]
                        for i in range(DB):
                            nc.tensor.matmul(
                                zh,
                                lhsT=wt[i][:, j * P:(j + 1) * P],
                                rhs=hc[i][:, hs:hs + hw],
                                start=(i == 0),
                                stop=False,
                            )
                        nc.tensor.matmul(
                            zh,
                            lhsT=identr,
                            rhs=ut[j][:, k + hs:k + hs + hw],
                            start=False,
                            stop=True,
                        )
                    r = rp.tile([P, w], F16, name="r", tag="r")
                    nc.scalar.activation(r[:], zt[:], AF.Relu)
                    nc.vector.tensor_tensor(
                        out=hn[j][:, cs:cs + w],
                        in0=ahs[gi, j][:],
                        in1=r[:],
                        op=ALU.add,
                    )

        # --- step 7 in 512-col chunks, output DMA per chunk ---
        k = KSIZE - 1
        hc = hb[k % 2]
        hn = hb[(k + 1) % 2]
        h8all = hball[(k + 1) % 2]
        ahs7 = {}
        for g in range(NG):
            for j in range(DB):
                ah = app.tile([P, MMN], F16, name="ah7", tag="ah")
                nc.vector.tensor_scalar(
                    out=ah[:],
                    in0=hc[j][:, g * MMN:(g + 1) * MMN],
                    scalar1=at[:, j:j + 1],
                    scalar2=None,
                    op0=ALU.mult,
                )
                ahs7[g, j] = ah
        # last 512-chunk tapers into 2x256 with descriptor-gens on different
        # engines, shortening the final relu->tt->dma drain chain.
        pieces = [(0, 0, MMN, nc.sync), (1, 0, MMN, nc.scalar),
                  (2, 0, MMN, nc.sync), (3, 0, MMN // 2, nc.sync),
                  (3, MMN // 2, MMN // 2, nc.scalar)]
        for g, off, w, eng in pieces:
            cs = g * MMN + off
            for j in range(DB):
                zt = zp.tile([P, w], F32, name="z7", tag="z")
                for i in range(DB):
                    nc.tensor.matmul(
                        zt[:],
                        lhsT=wt[i][:, j * P:(j + 1) * P],
                        rhs=hc[i][:, cs:cs + w],
                        start=(i == 0),
                        stop=False,
                    )
                nc.tensor.matmul(
                    zt[:],
                    lhsT=identr,
                    rhs=ut[j][:, k + cs:k + cs + w],
                    start=False,
                    stop=True,
                )
                r = rp.tile([P, w], F16, name="r7", tag="r")
                nc.scalar.activation(r[:], zt[:], AF.Relu)
                nc.vector.tensor_tensor(
                    out=hn[j][:, cs:cs + w],
                    in0=ahs7[g, j][:, off:off + w],
                    in1=r[:],
                    op=ALU.add,
                )
            # chunk-major store: rows [g*128, (g+1)*128) of out_d.
            # sync is idle in steady state; keep scalar free for relus.
            eng.dma_start(
                out_d.rearrange("(c p) f -> c p f", p=P)[g]
                     .rearrange("p (i s) -> p i s", i=DB)[:, :, off:off + w],
                h8all[:].rearrange("p (i c) -> p i c", i=DB)[
                    :, :, cs:cs + w],
            )

    nc.compile()
    return nc


def get_program():
    if "nc" not in _cache:
        _cache["nc"] = _build_program()
    return _cache["nc"]


def make_in_maps(x, weight, input_weight, bias, tau):
    x = np.asarray(x, dtype=np.float32)
    weight = np.asarray(weight, dtype=np.float32)
    input_weight = np.asarray(input_weight, dtype=np.float32)
    bias = np.asarray(bias, dtype=np.float32).reshape(1, D)
    tau = np.asarray(tau, dtype=np.float32).reshape(1, D)

    inv_tau = 1.0 / tau                       # (1, D)
    a = 1.0 - inv_tau
    wstar = (weight * inv_tau).astype(np.float32)          # scale columns
    winstar = (input_weight * inv_tau).astype(np.float32)
    bstar = (bias * inv_tau).astype(np.float32)
    # per-partition layout (P, DB): col j holds elems [j*P, (j+1)*P)
    bstar_t = bstar.reshape(DB, P).T
    a_t = a.reshape(DB, P).T
    ident = np.eye(P, dtype=np.float32)

    wb = np.concatenate(
        [winstar[0:P, :], winstar[P:D, :], wstar[0:P, :], wstar[P:D, :],
         ident], axis=1).astype(np.float16)
    cf = np.zeros((P, CF_COLS), np.float32)
    cf[:, 0:DB] = bstar_t
    cf[:, DB:2 * DB] = a_t

    shared = {
        "wblob": np.ascontiguousarray(wb),
        "constsf": np.ascontiguousarray(cf),
    }
    ins = []
    for b in range(NCORES):
        # xt[p, i*L + l] = x[b][l, i*128+p]
        xt = np.ascontiguousarray(
            x[b].T.reshape(DB, P, L).transpose(1, 0, 2).reshape(P, DB * L)
            .astype(np.float16))
        ins.append({"xt": xt, **shared})
    return ins


def kernel(x, weight, input_weight, bias, tau, ksize, _trace=False):
    assert int(ksize) == KSIZE
    nc = get_program()
    in_maps = make_in_maps(x, weight, input_weight, bias, tau)
    res = run_bass_kernel_spmd(
        nc, in_maps, core_ids=list(range(NCORES)), trace=_trace
    )
    outs = []
    for b in range(NCORES):
        od = np.asarray(res.results[b]["out"])  # (NG*P, DB*MMN) f16
        out_b = (od.reshape(NG, P, DB, MMN).transpose(0, 3, 2, 1)
                 .reshape(L, D))
        outs.append(out_b)
    out = np.stack(outs, axis=0)
    if _trace:
        _cache["last_results"] = res
    return out.astype(np.float32)


# revision 15
# speedup vs baseline: 1.0481x; 1.0384x over previous
"""Trainium2 Bass kernel for nn_LocalRNN: 8-step CTRNN over sliding windows.

Math:
  For each position l: h_{k+1} = a*h_k + relu(h_k @ W* + u*[l+k]),  h_0 = 0
  where a = 1 - 1/tau, W* = W * (1/tau) (columns), u* = Xp @ W_in* + b*,
  W_in* = W_in * (1/tau), b* = b * (1/tau).  Output = h_8 per position.
  (Uses relu(c*z) = c*relu(z) for c>0 to fold 1/tau into the weights, and
  the fact that the input projection is shared across overlapping windows.)

Sharding: batch dim (8) across the 8 NeuronCores, weights replicated.
On-chip layout is transposed ([d on partitions, positions on free dim]) so
matmuls contract d on the partition axis; the host uploads x pre-transposed
and swizzled, and un-swizzles the chunk-major output (layout marshalling
only, off the measured path).

All matmul/DVE operands are fp16 (PSUM accumulation stays fp32): enables
FWL weight loads on PE, 2x DVE mode for the recurrence, and halves DMA.
"""

import numpy as np
from contextlib import ExitStack

import concourse.bass as bass
import concourse.tile as tile
from concourse import bacc, mybir
from concourse.bass_utils import run_bass_kernel_spmd

B, L, D, KSIZE = 8, 2048, 256, 8
P = 128
NCORES = 8
MMN = 512                    # matmul moving free dim (PSUM bank limit)
WCH = 1024                   # wide chunk for ACT/DVE elementwise ops
NW = L // WCH                # 2
NG = L // MMN                # 4 groups of 512
UCOLS = L + KSIZE - 1        # 2055
PAD = KSIZE - 1              # 7
DB = D // P                  # 2 d-blocks
F32 = mybir.dt.float32
F16 = mybir.dt.float16
AF = mybir.ActivationFunctionType
ALU = mybir.AluOpType

# fp16 weights blob: wint0|wint1|wt0|wt1|identr
WB_COLS = 4 * D + P
WB_ID = 4 * D
# fp32 consts blob: bst (DB) | at (DB) | spare
CF_COLS = 16
N_WARM = 18
_cache = {}


def _build_program():
    nc = bacc.Bacc(
        "TRN2",
        target_bir_lowering=False,
        debug=False,
        enable_asserts=False,
        num_devices=NCORES,
    )
    # x uploaded pre-transposed+swizzled: row p, cols (i, l): x[l, i*128+p]
    x_d = nc.dram_tensor("xt", (P, DB * L), F16, kind="ExternalInput").ap()
    wb_d = nc.dram_tensor("wblob", (P, WB_COLS), F16, kind="ExternalInput").ap()
    cf_d = nc.dram_tensor("constsf", (P, CF_COLS), F32, kind="ExternalInput").ap()
    # output chunk-major: rows (c, p), cols (i, s): h8[i*128+p, c*512+s]
    out_d = nc.dram_tensor("out", (NG * P, DB * MMN), F16, kind="ExternalOutput").ap()

    with tile.TileContext(nc) as tc, ExitStack() as ctx:
        consts = ctx.enter_context(tc.tile_pool(name="consts", bufs=1))
        big = ctx.enter_context(tc.tile_pool(name="big", bufs=1))
        rp = ctx.enter_context(tc.tile_pool(name="rp", bufs=3))
        app = ctx.enter_context(tc.tile_pool(name="app", bufs=8))
        # single PSUM pool, all tags share slots: [128,1024] slot = 2 banks,
        # bufs=4 -> 8 banks
        zp = ctx.enter_context(tc.tile_pool(name="zp", bufs=4, space="PSUM"))

        # --- PE warmup on a memset dummy: starts right after the preamble
        # (no DMA dependency) and rides the HAM ramp while inputs land.
        dummy = big.tile([P, MMN], F16, name="dummy")
        nc.vector.memset(dummy[:], 0.0)
        warm = zp.tile([P, MMN], F32, name="warm", tag="z")
        for _ in range(N_WARM):
            nc.tensor.matmul(warm[:], lhsT=dummy[:, 0:P], rhs=dummy[:],
                             start=True, stop=True)

        # --- constants ---
        wb = consts.tile([P, WB_COLS], F16, name="wb")
        cf = consts.tile([P, CF_COLS], F32, name="cf")
        wint = [wb[:, i * D:(i + 1) * D] for i in range(DB)]
        wt = [wb[:, 2 * D + i * D:2 * D + (i + 1) * D] for i in range(DB)]
        identr = wb[:, WB_ID:WB_ID + P]
        bst = cf[:, 0:DB]
        at = cf[:, DB:2 * DB]

        # --- persistent buffers ---
        # x in 4 per-position-quarter tiles (both d-blocks each): each
        # 0.25MB piece's DMA-completion semaphore posts ~2.5us after its
        # transfer ends, so smaller first pieces unblock the u projection
        # earlier.
        xth = [big.tile([P, DB * MMN], F16, name=f"xth{q}") for q in range(4)]
        ut = [big.tile([P, UCOLS], F16, name=f"ut{i}") for i in range(DB)]
        hball = [big.tile([P, DB * L], F16, name=f"hb{s}") for s in range(2)]
        hb = [[hball[s][:, i * L:(i + 1) * L] for i in range(DB)]
              for s in range(2)]
        h1 = hb[1]

        # --- input DMAs. Only sync+scalar have HWDGE; each dma_start costs
        # ~700ns of descriptor-gen on its sequencer. x pieces in order on
        # sync (gate the u projection); cf + weights on scalar.
        def xdma(eng, q):
            eng.dma_start(
                xth[q][:].rearrange("p (i c) -> p i c", i=DB),
                x_d.rearrange("p (i c) -> p i c", i=DB)[
                    :, :, q * MMN:(q + 1) * MMN],
            )
        nc.scalar.dma_start(cf[:], cf_d[:, :])
        xdma(nc.sync, 0)
        nc.scalar.dma_start(wb[:], wb_d[:, :])
        xdma(nc.sync, 1)
        xdma(nc.scalar, 2)
        xdma(nc.sync, 3)

        # u pad cols (also warms the ACT table early): u[:, :7] = b*
        for j in range(DB):
            nc.scalar.activation(
                ut[j][:, 0:PAD], cf[:, 0:PAD],
                AF.Identity, bias=bst[:, j:j + 1], scale=0.0,
            )

        # --- u projection in 512-col pieces, pipelined against the x DMA
        # pieces. ut = z + b* split across ACT (even pieces) and DVE (odd);
        # h1 = relu(ut) runs on DVE at 4x (fp16 all-SBUF tensor_scalar).
        for q in range(4):
            for j in range(DB):
                zt = zp.tile([P, MMN], F32, name="zu", tag="z")
                for i in range(DB):
                    nc.tensor.matmul(
                        zt[:],
                        lhsT=wint[i][:, j * P:(j + 1) * P],
                        rhs=xth[q][:, i * MMN:(i + 1) * MMN],
                        start=(i == 0),
                        stop=(i == DB - 1),
                    )
                uo = PAD + q * MMN
                if q % 2 == 0:
                    nc.scalar.activation(
                        ut[j][:, uo:uo + MMN], zt[:],
                        AF.Identity, bias=bst[:, j:j + 1], scale=1.0,
                    )
                else:
                    nc.vector.tensor_scalar(
                        out=ut[j][:, uo:uo + MMN],
                        in0=zt[:],
                        scalar1=bst[:, j:j + 1],
                        scalar2=None,
                        op0=ALU.add,
                    )
                # h1 positions [q*512, (q+1)*512) = relu(ut[same cols])
                nc.vector.tensor_scalar(
                    out=h1[j][:, q * MMN:(q + 1) * MMN],
                    in0=ut[j][:, q * MMN:(q + 1) * MMN],
                    scalar1=0.0,
                    scalar2=None,
                    op0=ALU.max,
                )

        # --- steps 1..6 (wide chunks; matmuls in <=512 sub-tiles). Step 6
        # tapers its tail (1024/768/256) so the final two-step drain chain
        # into step 7's last piece runs on short tiles.
        def step_groups(k):
            return [(c * WCH, WCH) for c in range(NW)]

        def mm_halves(w):
            out, off = [], 0
            while off < w:
                m = min(MMN, w - off)
                out.append((off, m))
                off += m
            return out

        for k in range(1, KSIZE - 1):
            hc = hb[k % 2]
            hn = hb[(k + 1) % 2]
            groups = step_groups(k)
            # a*h precomputed on DVE (tensor_scalar runs 4x on fp16 SBUF)
            # while PE does the step's matmuls; the post-relu combine is a
            # 2x tensor_tensor. scalar_tensor_tensor would run 1x.
            ahs = {}
            for gi, (cs, w) in enumerate(groups):
                for j in range(DB):
                    ah = app.tile([P, w], F16, name="ah", tag="ah")
                    nc.vector.tensor_scalar(
                        out=ah[:],
                        in0=hc[j][:, cs:cs + w],
                        scalar1=at[:, j:j + 1],
                        scalar2=None,
                        op0=ALU.mult,
                    )
                    ahs[gi, j] = ah
            for gi, (cs, w) in enumerate(groups):
                for j in range(DB):
                    zt = zp.tile([P, w], F32, name="zt", tag="z")
                    for hs0, hw in mm_halves(w):
                        hs = cs + hs0
                        zh = zt[:, hs0:hs0 + hw]
                        for i in range(DB):
                            nc.tensor.matmul(
                                zh,
                                lhsT=wt[i][:, j * P:(j + 1) * P],
                                rhs=hc[i][:, hs:hs + hw],
                                start=(i == 0),
                                stop=False,
                            )
                        nc.tensor.matmul(
                            zh,
                            lhsT=identr,
                            rhs=ut[j][:, k + hs:k + hs + hw],
                            start=False,
                            stop=True,
                        )
                    r = rp.tile([P, w], F16, name="r", tag="r")
                    nc.scalar.activation(r[:], zt[:], AF.Relu)
                    nc.vector.tensor_tensor(
                        out=hn[j][:, cs:cs + w],
                        in0=ahs[gi, j][:],
                        in1=r[:],
                        op=ALU.add,
                    )

        # --- step 7 in 512-col chunks, output DMA per chunk ---
        k = KSIZE - 1
        hc = hb[k % 2]
        hn = hb[(k + 1) % 2]
        h8all = hball[(k + 1) % 2]
        ahs7 = {}
        for g in range(NG):
            for j in range(DB):
                ah = app.tile([P, MMN], F16, name="ah7", tag="ah")
                nc.vector.tensor_scalar(
                    out=ah[:],
                    in0=hc[j][:, g * MMN:(g + 1) * MMN],
                    scalar1=at[:, j:j + 1],
                    scalar2=None,
                    op0=ALU.mult,
                )
                ahs7[g, j] = ah
        # last 512-chunk tapers into 2x256 with descriptor-gens on different
        # engines, shortening the final relu->tt->dma drain chain.
        pieces = [(0, 0, MMN, nc.sync), (1, 0, MMN, nc.scalar),
                  (2, 0, MMN, nc.sync), (3, 0, MMN // 2, nc.sync),
                  (3, MMN // 2, MMN // 2, nc.scalar)]
        for g, off, w, eng in pieces:
            cs = g * MMN + off
            for j in range(DB):
                zt = zp.tile([P, w], F32, name="z7", tag="z")
                for i in range(DB):
                    nc.tensor.matmul(
                        zt[:],
                        lhsT=wt[i][:, j * P:(j + 1) * P],
                        rhs=hc[i][:, cs:cs + w],
                        start=(i == 0),
                        stop=False,
                    )
                nc.tensor.matmul(
                    zt[:],
                    lhsT=identr,
                    rhs=ut[j][:, k + cs:k + cs + w],
                    start=False,
                    stop=True,
                )
                r = rp.tile([P, w], F16, name="r7", tag="r")
                nc.scalar.activation(r[:], zt[:], AF.Relu)
                nc.vector.tensor_tensor(
                    out=hn[j][:, cs:cs + w],
                    in0=ahs7[g, j][:, off:off + w],
                    in1=r[:],
                    op=ALU.add,
                )
            # chunk-major store: rows [g*128, (g+1)*128) of out_d.
            # sync is idle in steady state; keep scalar free for relus.
            eng.dma_start(
                out_d.rearrange("(c p) f -> c p f", p=P)[g]
                     .rearrange("p (i s) -> p i s", i=DB)[:, :, off:off + w],
                h8all[:].rearrange("p (i c) -> p i c", i=DB)[
                    :, :, cs:cs + w],
            )

    nc.compile()
    return nc


def get_program():
    if "nc" not in _cache:
        _cache["nc"] = _build_program()
    return _cache["nc"]


def make_in_maps(x, weight, input_weight, bias, tau):
    x = np.asarray(x, dtype=np.float32)
    weight = np.asarray(weight, dtype=np.float32)
    input_weight = np.asarray(input_weight, dtype=np.float32)
    bias = np.asarray(bias, dtype=np.float32).reshape(1, D)
    tau = np.asarray(tau, dtype=np.float32).reshape(1, D)

    inv_tau = 1.0 / tau                       # (1, D)
    a = 1.0 - inv_tau
    wstar = (weight * inv_tau).astype(np.float32)          # scale columns
    winstar = (input_weight * inv_tau).astype(np.float32)
    bstar = (bias * inv_tau).astype(np.float32)
    # per-partition layout (P, DB): col j holds elems [j*P, (j+1)*P)
    bstar_t = bstar.reshape(DB, P).T
    a_t = a.reshape(DB, P).T
    ident = np.eye(P, dtype=np.float32)

    wb = np.concatenate(
        [winstar[0:P, :], winstar[P:D, :], wstar[0:P, :], wstar[P:D, :],
         ident], axis=1).astype(np.float16)
    cf = np.zeros((P, CF_COLS), np.float32)
    cf[:, 0:DB] = bstar_t
    cf[:, DB:2 * DB] = a_t

    shared = {
        "wblob": np.ascontiguousarray(wb),
        "constsf": np.ascontiguousarray(cf),
    }
    ins = []
    for b in range(NCORES):
        # xt[p, i*L + l] = x[b][l, i*128+p]
        xt = np.ascontiguousarray(
            x[b].T.reshape(DB, P, L).transpose(1, 0, 2).reshape(P, DB * L)
            .astype(np.float16))
        ins.append({"xt": xt, **shared})
    return ins


def kernel(x, weight, input_weight, bias, tau, ksize, _trace=False):
    assert int(ksize) == KSIZE
    nc = get_program()
    in_maps = make_in_maps(x, weight, input_weight, bias, tau)
    res = run_bass_kernel_spmd(
        nc, in_maps, core_ids=list(range(NCORES)), trace=_trace
    )
    outs = []
    for b in range(NCORES):
        od = np.asarray(res.results[b]["out"])  # (NG*P, DB*MMN) f16
        out_b = (od.reshape(NG, P, DB, MMN).transpose(0, 3, 2, 1)
                 .reshape(L, D))
        outs.append(out_b)
    out = np.stack(outs, axis=0)
    if _trace:
        _cache["last_results"] = res
    return out.astype(np.float32)


# revision 20
# speedup vs baseline: 1.0597x; 1.0111x over previous
"""Trainium2 Bass kernel for nn_LocalRNN: 8-step CTRNN over sliding windows.

Math:
  For each position l: h_{k+1} = a*h_k + relu(h_k @ W* + u*[l+k]),  h_0 = 0
  where a = 1 - 1/tau, W* = W * (1/tau) (columns), u* = Xp @ W_in* + b*,
  W_in* = W_in * (1/tau), b* = b * (1/tau).  Output = h_8 per position.
  (Uses relu(c*z) = c*relu(z) for c>0 to fold 1/tau into the weights, and
  the fact that the input projection is shared across overlapping windows.)

Sharding: batch dim (8) across the 8 NeuronCores, weights replicated.
On-chip layout is transposed ([d on partitions, positions on free dim]) so
matmuls contract d on the partition axis; the host uploads x pre-transposed
and swizzled, and un-swizzles the chunk-major output (layout marshalling
only, off the measured path).

All matmul/DVE operands are fp16 (PSUM accumulation stays fp32): enables
FWL weight loads on PE, 2x DVE mode for the recurrence, and halves DMA.
"""

import numpy as np
from contextlib import ExitStack

import concourse.bass as bass
import concourse.tile as tile
from concourse import bacc, mybir
from concourse.bass_utils import run_bass_kernel_spmd

B, L, D, KSIZE = 8, 2048, 256, 8
P = 128
NCORES = 8
MMN = 512                    # matmul moving free dim (PSUM bank limit)
WCH = 1024                   # wide chunk for ACT/DVE elementwise ops
NW = L // WCH                # 2
NG = L // MMN                # 4 groups of 512
UCOLS = L + KSIZE - 1        # 2055
PAD = KSIZE - 1              # 7
DB = D // P                  # 2 d-blocks
F32 = mybir.dt.float32
F16 = mybir.dt.float16
AF = mybir.ActivationFunctionType
ALU = mybir.AluOpType

# fp16 weights blob A: wint0|wint1|identr (gates the u projection);
# blob B: wt0|wt1 (needed one step later)
WBA_COLS = 2 * D + P
WBA_ID = 2 * D
WBB_COLS = 2 * D
# fp32 consts blob: bst (DB) | at (DB) | spare
CF_COLS = 16
N_WARM = 10
_cache = {}


def _build_program():
    nc = bacc.Bacc(
        "TRN2",
        target_bir_lowering=False,
        debug=False,
        enable_asserts=False,
        num_devices=NCORES,
    )
    # x uploaded pre-transposed+swizzled: row p, cols (i, l): x[l, i*128+p]
    x_d = nc.dram_tensor("xt", (P, DB * L), F16, kind="ExternalInput").ap()
    wba_d = nc.dram_tensor("wbloba", (P, WBA_COLS), F16, kind="ExternalInput").ap()
    wbb_d = nc.dram_tensor("wblobb", (P, WBB_COLS), F16, kind="ExternalInput").ap()
    cf_d = nc.dram_tensor("constsf", (P, CF_COLS), F32, kind="ExternalInput").ap()
    # output chunk-major: rows (c, p), cols (i, s): h8[i*128+p, c*512+s]
    out_d = nc.dram_tensor("out", (NG * P, DB * MMN), F16, kind="ExternalOutput").ap()

    with tile.TileContext(nc) as tc, ExitStack() as ctx:
        consts = ctx.enter_context(tc.tile_pool(name="consts", bufs=1))
        big = ctx.enter_context(tc.tile_pool(name="big", bufs=1))
        rp = ctx.enter_context(tc.tile_pool(name="rp", bufs=3))
        app = ctx.enter_context(tc.tile_pool(name="app", bufs=8))
        # single PSUM pool, all tags share slots: [128,1024] slot = 2 banks,
        # bufs=4 -> 8 banks
        zp = ctx.enter_context(tc.tile_pool(name="zp", bufs=4, space="PSUM"))

        # --- PE warmup on a memset dummy: starts right after the preamble
        # (no DMA dependency) and rides the HAM ramp while inputs land.
        dummy = big.tile([P, MMN], F16, name="dummy")
        nc.vector.memset(dummy[:], 0.0)
        warm = zp.tile([P, MMN], F32, name="warm", tag="z")
        for _ in range(N_WARM):
            nc.tensor.matmul(warm[:], lhsT=dummy[:, 0:P], rhs=dummy[:],
                             start=True, stop=True)

        # --- constants ---
        wba = consts.tile([P, WBA_COLS], F16, name="wba")
        wbb = consts.tile([P, WBB_COLS], F16, name="wbb")
        cf = consts.tile([P, CF_COLS], F32, name="cf")
        wint = [wba[:, i * D:(i + 1) * D] for i in range(DB)]
        wt = [wbb[:, i * D:(i + 1) * D] for i in range(DB)]
        identr = wba[:, WBA_ID:WBA_ID + P]
        bst = cf[:, 0:DB]
        at = cf[:, DB:2 * DB]

        # --- persistent buffers ---
        # x in 4 per-position-quarter tiles (both d-blocks each): each
        # 0.25MB piece's DMA-completion semaphore posts ~2.5us after its
        # transfer ends, so smaller first pieces unblock the u projection
        # earlier.
        xth = [big.tile([P, DB * MMN], F16, name=f"xth{q}") for q in range(4)]
        ut = [big.tile([P, UCOLS], F16, name=f"ut{i}") for i in range(DB)]
        hball = [big.tile([P, DB * L], F16, name=f"hb{s}") for s in range(2)]
        hb = [[hball[s][:, i * L:(i + 1) * L] for i in range(DB)]
              for s in range(2)]
        h1 = hb[1]

        # --- input DMAs. Only sync+scalar have HWDGE; each dma_start costs
        # ~700ns of descriptor-gen on its sequencer. x pieces in order on
        # sync (gate the u projection); cf + weights on scalar.
        def xdma(eng, q):
            eng.dma_start(
                xth[q][:].rearrange("p (i c) -> p i c", i=DB),
                x_d.rearrange("p (i c) -> p i c", i=DB)[
                    :, :, q * MMN:(q + 1) * MMN],
            )
        nc.scalar.dma_start(wba[:], wba_d[:, :])
        xdma(nc.sync, 0)
        nc.scalar.dma_start(cf[:], cf_d[:, :])
        xdma(nc.sync, 1)
        nc.scalar.dma_start(wbb[:], wbb_d[:, :])
        xdma(nc.sync, 2)
        xdma(nc.sync, 3)

        # u pad cols (also warms the ACT table early): u[:, :7] = b*
        for j in range(DB):
            nc.scalar.activation(
                ut[j][:, 0:PAD], cf[:, 0:PAD],
                AF.Identity, bias=bst[:, j:j + 1], scale=0.0,
            )

        # --- u projection in 512-col pieces, pipelined against the x DMA
        # pieces. ut = z + b* split across ACT (even pieces) and DVE (odd);
        # h1 = relu(ut) runs on DVE at 4x (fp16 all-SBUF tensor_scalar).
        for q in range(4):
            for j in range(DB):
                zt = zp.tile([P, MMN], F32, name="zu", tag="z")
                for i in range(DB):
                    nc.tensor.matmul(
                        zt[:],
                        lhsT=wint[i][:, j * P:(j + 1) * P],
                        rhs=xth[q][:, i * MMN:(i + 1) * MMN],
                        start=(i == 0),
                        stop=(i == DB - 1),
                    )
                uo = PAD + q * MMN
                if q % 2 == 0:
                    nc.scalar.activation(
                        ut[j][:, uo:uo + MMN], zt[:],
                        AF.Identity, bias=bst[:, j:j + 1], scale=1.0,
                    )
                else:
                    nc.vector.tensor_scalar(
                        out=ut[j][:, uo:uo + MMN],
                        in0=zt[:],
                        scalar1=bst[:, j:j + 1],
                        scalar2=None,
                        op0=ALU.add,
                    )
                # h1 positions [q*512, (q+1)*512) = relu(ut[same cols])
                nc.vector.tensor_scalar(
                    out=h1[j][:, q * MMN:(q + 1) * MMN],
                    in0=ut[j][:, q * MMN:(q + 1) * MMN],
                    scalar1=0.0,
                    scalar2=None,
                    op0=ALU.max,
                )

        # --- steps 1..6 (wide chunks; matmuls in <=512 sub-tiles). Step 6
        # tapers its tail (1024/768/256) so the final two-step drain chain
        # into step 7's last piece runs on short tiles.
        def step_groups(k):
            return [(c * WCH, WCH) for c in range(NW)]

        def mm_halves(w):
            out, off = [], 0
            while off < w:
                m = min(MMN, w - off)
                out.append((off, m))
                off += m
            return out

        for k in range(1, KSIZE - 1):
            hc = hb[k % 2]
            hn = hb[(k + 1) % 2]
            groups = step_groups(k)
            # a*h precomputed on DVE (tensor_scalar runs 4x on fp16 SBUF)
            # while PE does the step's matmuls; the post-relu combine is a
            # 2x tensor_tensor. scalar_tensor_tensor would run 1x.
            ahs = {}
            for gi, (cs, w) in enumerate(groups):
                for j in range(DB):
                    ah = app.tile([P, w], F16, name="ah", tag="ah")
                    nc.vector.tensor_scalar(
                        out=ah[:],
                        in0=hc[j][:, cs:cs + w],
                        scalar1=at[:, j:j + 1],
                        scalar2=None,
                        op0=ALU.mult,
                    )
                    ahs[gi, j] = ah
            for gi, (cs, w) in enumerate(groups):
                for j in range(DB):
                    zt = zp.tile([P, w], F32, name="zt", tag="z")
                    for hs0, hw in mm_halves(w):
                        hs = cs + hs0
                        zh = zt[:, hs0:hs0 + hw]
                        for i in range(DB):
                            nc.tensor.matmul(
                                zh,
                                lhsT=wt[i][:, j * P:(j + 1) * P],
                                rhs=hc[i][:, hs:hs + hw],
                                start=(i == 0),
                                stop=False,
                            )
                        nc.tensor.matmul(
                            zh,
                            lhsT=identr,
                            rhs=ut[j][:, k + hs:k + hs + hw],
                            start=False,
                            stop=True,
                        )
                    r = rp.tile([P, w], F16, name="r", tag="r")
                    nc.scalar.activation(r[:], zt[:], AF.Relu)
                    nc.vector.tensor_tensor(
                        out=hn[j][:, cs:cs + w],
                        in0=ahs[gi, j][:],
                        in1=r[:],
                        op=ALU.add,
                    )

        # --- step 7 in 512-col chunks, output DMA per chunk ---
        k = KSIZE - 1
        hc = hb[k % 2]
        hn = hb[(k + 1) % 2]
        h8all = hball[(k + 1) % 2]
        ahs7 = {}
        for g in range(NG):
            for j in range(DB):
                ah = app.tile([P, MMN], F16, name="ah7", tag="ah")
                nc.vector.tensor_scalar(
                    out=ah[:],
                    in0=hc[j][:, g * MMN:(g + 1) * MMN],
                    scalar1=at[:, j:j + 1],
                    scalar2=None,
                    op0=ALU.mult,
                )
                ahs7[g, j] = ah
        # last 512-chunk tapers into 2x256 with descriptor-gens on different
        # engines, shortening the final relu->tt->dma drain chain.
        pieces = [(0, 0, MMN, nc.sync), (1, 0, MMN, nc.scalar),
                  (2, 0, MMN, nc.sync), (3, 0, MMN // 2, nc.sync),
                  (3, MMN // 2, MMN // 2, nc.scalar)]
        for g, off, w, eng in pieces:
            cs = g * MMN + off
            for j in range(DB):
                zt = zp.tile([P, w], F32, name="z7", tag="z")
                for i in range(DB):
                    nc.tensor.matmul(
                        zt[:],
                        lhsT=wt[i][:, j * P:(j + 1) * P],
                        rhs=hc[i][:, cs:cs + w],
                        start=(i == 0),
                        stop=False,
                    )
                nc.tensor.matmul(
                    zt[:],
                    lhsT=identr,
                    rhs=ut[j][:, k + cs:k + cs + w],
                    start=False,
                    stop=True,
                )
                r = rp.tile([P, w], F16, name="r7", tag="r")
                nc.scalar.activation(r[:], zt[:], AF.Relu)
                nc.vector.tensor_tensor(
                    out=hn[j][:, cs:cs + w],
                    in0=ahs7[g, j][:, off:off + w],
                    in1=r[:],
                    op=ALU.add,
                )
            # chunk-major store: rows [g*128, (g+1)*128) of out_d.
            # sync is idle in steady state; keep scalar free for relus.
            eng.dma_start(
                out_d.rearrange("(c p) f -> c p f", p=P)[g]
                     .rearrange("p (i s) -> p i s", i=DB)[:, :, off:off + w],
                h8all[:].rearrange("p (i c) -> p i c", i=DB)[
                    :, :, cs:cs + w],
            )

    nc.compile()
    return nc


def get_program():
    if "nc" not in _cache:
        _cache["nc"] = _build_program()
    return _cache["nc"]


def make_in_maps(x, weight, input_weight, bias, tau):
    x = np.asarray(x, dtype=np.float32)
    weight = np.asarray(weight, dtype=np.float32)
    input_weight = np.asarray(input_weight, dtype=np.float32)
    bias = np.asarray(bias, dtype=np.float32).reshape(1, D)
    tau = np.asarray(tau, dtype=np.float32).reshape(1, D)

    inv_tau = 1.0 / tau                       # (1, D)
    a = 1.0 - inv_tau
    wstar = (weight * inv_tau).astype(np.float32)          # scale columns
    winstar = (input_weight * inv_tau).astype(np.float32)
    bstar = (bias * inv_tau).astype(np.float32)
    # per-partition layout (P, DB): col j holds elems [j*P, (j+1)*P)
    bstar_t = bstar.reshape(DB, P).T
    a_t = a.reshape(DB, P).T
    ident = np.eye(P, dtype=np.float32)

    wba = np.concatenate(
        [winstar[0:P, :], winstar[P:D, :], ident], axis=1).astype(np.float16)
    wbb = np.concatenate(
        [wstar[0:P, :], wstar[P:D, :]], axis=1).astype(np.float16)
    cf = np.zeros((P, CF_COLS), np.float32)
    cf[:, 0:DB] = bstar_t
    cf[:, DB:2 * DB] = a_t

    shared = {
        "wbloba": np.ascontiguousarray(wba),
        "wblobb": np.ascontiguousarray(wbb),
        "constsf": np.ascontiguousarray(cf),
    }
    ins = []
    for b in range(NCORES):
        # xt[p, i*L + l] = x[b][l, i*128+p]
        xt = np.ascontiguousarray(
            x[b].T.reshape(DB, P, L).transpose(1, 0, 2).reshape(P, DB * L)
            .astype(np.float16))
        ins.append({"xt": xt, **shared})
    return ins


def kernel(x, weight, input_weight, bias, tau, ksize, _trace=False):
    assert int(ksize) == KSIZE
    nc = get_program()
    in_maps = make_in_maps(x, weight, input_weight, bias, tau)
    res = run_bass_kernel_spmd(
        nc, in_maps, core_ids=list(range(NCORES)), trace=_trace
    )
    outs = []
    for b in range(NCORES):
        od = np.asarray(res.results[b]["out"])  # (NG*P, DB*MMN) f16
        out_b = (od.reshape(NG, P, DB, MMN).transpose(0, 3, 2, 1)
                 .reshape(L, D))
        outs.append(out_b)
    out = np.stack(outs, axis=0)
    if _trace:
        _cache["last_results"] = res
    return out.astype(np.float32)
